# revision 16
# baseline (speedup 1.0000x reference)
"""2-layer GCN (gnn_message_passing) on 8 Trainium2 NeuronCores.

Strategy: dest-shard nodes across 8 cores (12500/core). Per layer:
  support = X @ W on PE (fp16 in, f32 psum) -> AllGather fp16 support table
  -> per dest-block of 128 rows: dma_gather source rows by column index
  (edges pre-bucketed by (core, dest block, table bank) on host, int16
  bank-local indices), scatter into the block via one-hot matmul
  accumulated in PSUM.  ReLU between layers; h1 kept SBUF-resident and
  PE-transposed for layer-2 support matmul.
"""
import sys
sys.path.insert(0, "/opt/trn_rl_repo")

import numpy as np
from contextlib import ExitStack

import concourse.bass as bass
import concourse.bacc as bacc
import concourse.tile as tile
from concourse import bass_utils
from concourse import mybir
from concourse.library_config import mlp

PADVAL = 200.0
GMAX = 4  # max 128-idx chunks per dma_gather call (HW-validated at 512 idx)


class Config:
    def __init__(self, n=100000, in_dim=256, hid=128, out_dim=64, ncore=8, nbank=4):
        self.N = n
        self.IN = in_dim
        self.HID = hid
        self.OUT = out_dim
        self.NCORE = ncore
        self.NPC = n // ncore
        assert self.NPC * ncore == n
        self.NB = (self.NPC + 127) // 128
        self.NPP = self.NB * 128
        self.NT = ncore * self.NPP
        self.NBANK = nbank
        assert self.NT % nbank == 0
        self.RPB = self.NT // nbank
        assert self.RPB <= 32767
        self.KT = in_dim // 128
        assert self.KT * 128 == in_dim


CFG = Config()


def prep_edges(cfg, edge_index):
    """Bucket edges by (dest core, dest block, src bank); pad each bucket to a
    multiple of 128. Returns per-core int16 gather indices (16-wrapped layout
    for dma_gather), per-core fp16 dest-row-in-block table, and the per-cell
    chunk counts (uniform across cores)."""
    c = cfg
    row = np.asarray(edge_index[0], dtype=np.int64)
    col = np.asarray(edge_index[1], dtype=np.int64)
    core = row // c.NPC
    rl = row - core * c.NPC
    blk = rl // 128
    rib = (rl % 128).astype(np.float16)
    tcol = (col // c.NPC) * c.NPP + (col % c.NPC)
    bank = tcol // c.RPB
    tloc = (tcol - bank * c.RPB).astype(np.int16)

    key = (core * c.NB + blk) * c.NBANK + bank
    order = np.argsort(key, kind="stable")
    key_s = key[order]
    tloc_s = tloc[order]
    rib_s = rib[order]
    ncell = c.NCORE * c.NB * c.NBANK
    counts = np.bincount(key_s, minlength=ncell).reshape(c.NCORE, c.NB, c.NBANK)
    starts = np.concatenate([[0], np.cumsum(counts.reshape(-1))]).reshape(-1)

    chunks = -(-counts.max(axis=0) // 128)  # (NB, NBANK) chunks per cell
    nch = int(chunks.sum())
    cw = nch * 8  # colidx columns (per chunk: 128 idx 16-wrapped -> 8 cols)

    colidx = np.zeros((c.NCORE, 128, cw), np.int16)
    rowloc = np.full((c.NCORE, 128, nch), PADVAL, np.float16)

    qoff = np.zeros((c.NB, c.NBANK), np.int64)  # chunk offset of each cell
    q = 0
    for b in range(c.NB):
        for k in range(c.NBANK):
            qoff[b, k] = q
            q += chunks[b, k]

    for cc in range(c.NCORE):
        for b in range(c.NB):
            for k in range(c.NBANK):
                nk = int(chunks[b, k])
                if nk == 0:
                    continue
                ki = (cc * c.NB + b) * c.NBANK + k
                s, e = starts[ki], starts[ki + 1]
                cnt = e - s
                cap = nk * 128
                tl = np.zeros(cap, np.int16)
                tl[:cnt] = tloc_s[s:e]
                rb = np.full(cap, PADVAL, np.float16)
                rb[:cnt] = rib_s[s:e]
                qo = int(qoff[b, k])
                colidx[cc][:, qo * 8:(qo + nk) * 8] = np.tile(
                    tl.reshape(-1, 16).T, (8, 1))
                rowloc[cc][:, qo:qo + nk] = rb.reshape(nk, 128).T
    return colidx, rowloc, chunks, qoff, nch, cw


def build(cfg, chunks, qoff, nch, cw):
    c = cfg
    nc = bacc.Bacc(None, target_bir_lowering=False, debug=False,
                   num_devices=c.NCORE, name="gcn", num_swdge_queues=4)
    f16, f32, i16 = mybir.dt.float16, mybir.dt.float32, mybir.dt.int16

    xT = nc.dram_tensor("xT", (c.IN, c.NPP), f16, kind="ExternalInput")
    w1 = nc.dram_tensor("w1", (c.IN, c.HID), f16, kind="ExternalInput")
    w2p = nc.dram_tensor("w2p", (c.HID, 128), f16, kind="ExternalInput")
    iota = nc.dram_tensor("iota", (128, 128), f16, kind="ExternalInput")
    ident = nc.dram_tensor("ident", (128, 128), f16, kind="ExternalInput")
    colidx = nc.dram_tensor("colidx", (128, cw), i16, kind="ExternalInput")
    rowloc = nc.dram_tensor("rowloc", (128, nch), f16, kind="ExternalInput")
    out = nc.dram_tensor("out", (c.NPC, c.OUT), f32, kind="ExternalOutput")

    sup1 = nc.dram_tensor("sup1", (c.NPP, c.HID), f16, kind="Internal")
    tab1 = nc.dram_tensor("tab1", (c.NT, c.HID), f16, kind="Internal",
                          addr_space="Shared")
    sup2 = nc.dram_tensor("sup2", (c.NPP, 128), f16, kind="Internal")
    tab2 = nc.dram_tensor("tab2", (c.NT, 128), f16, kind="Internal",
                          addr_space="Shared")

    groups = [list(range(c.NCORE))]
    iseq = mybir.AluOpType.is_equal

    with ExitStack() as ctx:
        tc = ctx.enter_context(tile.TileContext(nc))
        nc.gpsimd.load_library(mlp)
        cpool = ctx.enter_context(tc.tile_pool(name="const", bufs=1))
        iota_sb = cpool.tile((128, 128), f16, tag="iota")
        nc.sync.dma_start(iota_sb[:], iota[:])
        ident_sb = cpool.tile((128, 128), f16, tag="ident")
        nc.sync.dma_start(ident_sb[:], ident[:])
        w2_sb = cpool.tile((c.HID, 128), f16, tag="w2")
        nc.sync.dma_start(w2_sb[:], w2p[:])
        rowloc_sb = cpool.tile((128, nch), f16, tag="rowloc")
        nc.sync.dma_start(rowloc_sb[:], rowloc[:])
        h1T = cpool.tile((c.HID, c.NPP), f16, tag="h1T")

        # Phase A: support1 = X @ W1, per dest block
        with tc.tile_pool(name="pa", bufs=1) as pa, \
             tc.tile_pool(name="pas", bufs=3) as pas, \
             tc.tile_pool(name="psa", bufs=2, space="PSUM") as psa:
            xk = []
            w1k = []
            for k in range(c.KT):
                t = pa.tile((128, c.NPP), f16, tag=f"x{k}", name=f"xk{k}")
                nc.sync.dma_start(t[:], xT.ap()[k * 128:(k + 1) * 128, :])
                xk.append(t)
                t = pa.tile((128, c.HID), f16, tag=f"w{k}", name=f"w1k{k}")
                nc.sync.dma_start(t[:], w1.ap()[k * 128:(k + 1) * 128, :])
                w1k.append(t)
            for b in range(c.NB):
                ps = psa.tile((128, c.HID), f32, space="PSUM")
                for k in range(c.KT):
                    nc.tensor.matmul(ps[:], xk[k][:, b * 128:(b + 1) * 128],
                                     w1k[k][:], start=(k == 0),
                                     stop=(k == c.KT - 1))
                s1 = pas.tile((128, c.HID), f16)
                nc.vector.tensor_copy(s1[:], ps[:])
                nc.sync.dma_start(sup1.ap()[b * 128:(b + 1) * 128, :], s1[:])

        nc.gpsimd.collective_compute(
            "AllGather", mybir.AluOpType.bypass, replica_groups=groups,
            ins=[sup1.ap()], outs=[tab1.ap()])

        def scatter_layer(tab, width, emit):
            """Gather+scatter all dest blocks from table `tab` (row width
            `width` fp16); call emit(b, psum_ap) per finished block."""
            gq = [0]
            with tc.tile_pool(name="poh", bufs=2) as poh, \
                 tc.tile_pool(name="pg", bufs=6) as pg, \
                 tc.tile_pool(name="pix", bufs=4) as pix, \
                 tc.tile_pool(name="pso", bufs=2, space="PSUM") as pso:
                for b in range(c.NB):
                    wb = int(chunks[b].sum())
                    q0 = int(qoff[b, 0])
                    ps = pso.tile((128, width), f32, space="PSUM")
                    if wb == 0:
                        zt = pg.tile((128, width), f16)
                        nc.vector.memset(zt[:], 0.0)
                        nc.tensor.matmul(ps[:], ident_sb[:], zt[:],
                                         start=True, stop=True)
                        emit(b, ps)
                        continue
                    oh = poh.tile((128, wb, 128), f16)
                    nc.vector.tensor_tensor(
                        out=oh[:],
                        in0=rowloc_sb[:, q0:q0 + wb].unsqueeze(2)
                            .to_broadcast((128, wb, 128)),
                        in1=iota_sb[:].unsqueeze(1)
                            .to_broadcast((128, wb, 128)),
                        op=iseq)
                    qj = 0
                    for k in range(c.NBANK):
                        nk = int(chunks[b, k])
                        if nk == 0:
                            continue
                        qo = int(qoff[b, k])
                        for s0 in range(0, nk, GMAX):
                            sn = min(GMAX, nk - s0)
                            qs = qo + s0
                            idx_sb = pix.tile((128, sn * 8), i16)
                            nc.sync.dma_start(
                                idx_sb[:],
                                colidx.ap()[:, qs * 8:(qs + sn) * 8])
                            g = pg.tile((128, sn, 128), f16)
                            nc.gpsimd.dma_gather(
                                g[:], tab.ap()[k * c.RPB:(k + 1) * c.RPB],
                                idx_sb[:], sn * 128, sn * 128, 128,
                                queue_num=gq[0] % 4)
                            gq[0] += 1
                            for j in range(sn):
                                nc.tensor.matmul(ps[:], oh[:, qj, :],
                                                 g[:, j, 0:width],
                                                 start=(qj == 0),
                                                 stop=(qj == wb - 1))
                                qj += 1
                    emit(b, ps)

        # Phase C: layer-1 scatter, relu, transpose into h1T
        with tc.tile_pool(name="ph1", bufs=3) as ph1, \
             tc.tile_pool(name="pst", bufs=2, space="PSUM") as pst:
            def emit1(b, ps):
                h1b = ph1.tile((128, c.HID), f16)
                nc.vector.tensor_scalar_max(h1b[:], ps[:], 0.0)
                tp = pst.tile((c.HID, 128), f16, space="PSUM")
                nc.tensor.transpose(out=tp[:], in_=h1b[:],
                                    identity=ident_sb[:])
                nc.vector.tensor_copy(h1T[:, b * 128:(b + 1) * 128], tp[:])
            scatter_layer(tab1, c.HID, emit1)

        # Phase D: support2 = relu(h1) @ W2 (padded to 128 cols)
        with tc.tile_pool(name="pds", bufs=3) as pds, \
             tc.tile_pool(name="psd", bufs=2, space="PSUM") as psd:
            for b in range(c.NB):
                ps = psd.tile((128, 128), f32, space="PSUM")
                nc.tensor.matmul(ps[:], h1T[:, b * 128:(b + 1) * 128],
                                 w2_sb[:], start=True, stop=True)
                s2 = pds.tile((128, 128), f16)
                nc.vector.tensor_copy(s2[:], ps[:])
                nc.sync.dma_start(sup2.ap()[b * 128:(b + 1) * 128, :], s2[:])

        nc.gpsimd.collective_compute(
            "AllGather", mybir.AluOpType.bypass, replica_groups=groups,
            ins=[sup2.ap()], outs=[tab2.ap()])

        # Phase E: layer-2 scatter -> output
        with tc.tile_pool(name="po", bufs=3) as po:
            def emit2(b, ps):
                ob = po.tile((128, c.OUT), f32)
                nc.vector.tensor_copy(ob[:], ps[:])
                rows = min(128, c.NPC - b * 128)
                nc.sync.dma_start(out.ap()[b * 128:b * 128 + rows, :],
                                  ob[0:rows, :])
            scatter_layer(tab2, c.OUT, emit2)

    nc.compile()
    return nc


def make_inputs(cfg, features, edge_index, W1, W2):
    c = cfg
    colidx, rowloc, chunks, qoff, nch, cw = prep_edges(cfg, edge_index)
    iota2d = np.broadcast_to(np.arange(128, dtype=np.float16),
                             (128, 128)).copy()
    ident = np.eye(128, dtype=np.float16)
    w1 = np.ascontiguousarray(np.asarray(W1, np.float16))
    w2p = np.zeros((c.HID, 128), np.float16)
    w2p[:, :c.OUT] = np.asarray(W2, np.float16)
    in_maps = []
    for cc in range(c.NCORE):
        xc = np.asarray(features[cc * c.NPC:(cc + 1) * c.NPC], np.float32)
        xt = np.zeros((c.IN, c.NPP), np.float16)
        xt[:, :c.NPC] = xc.T.astype(np.float16)
        in_maps.append({
            "xT": np.ascontiguousarray(xt),
            "w1": w1, "w2p": w2p, "iota": iota2d, "ident": ident,
            "colidx": np.ascontiguousarray(colidx[cc]),
            "rowloc": np.ascontiguousarray(rowloc[cc]),
        })
    return in_maps, chunks, qoff, nch, cw


_LAST_NC = None


def kernel(features, edge_index, W1, W2):
    global _LAST_NC
    cfg = CFG
    in_maps, chunks, qoff, nch, cw = make_inputs(
        cfg, features, edge_index, W1, W2)
    nc = build(cfg, chunks, qoff, nch, cw)
    _LAST_NC = nc
    res = bass_utils.run_bass_kernel_spmd(
        nc, in_maps, core_ids=list(range(cfg.NCORE)))
    return np.concatenate(
        [res.results[cc]["out"] for cc in range(cfg.NCORE)], axis=0)



# revision 27
# speedup vs baseline: 1.9880x; 1.9880x over previous
"""2-layer GCN (gnn_message_passing) on 8 Trainium2 NeuronCores.

Strategy: dest-shard nodes across 8 cores (12500/core). Per layer:
  support = X @ W on PE (fp16 in, f32 psum) -> AllGather fp16 support table
  -> per dest-block of 128 rows: dma_gather source rows by column index
  (edges pre-bucketed by (core, dest block, table bank) on host, int16
  bank-local indices), scatter into the block via one-hot matmul
  accumulated in PSUM.  ReLU between layers; h1 kept SBUF-resident and
  PE-transposed for layer-2 support matmul.
"""
import sys
sys.path.insert(0, "/opt/trn_rl_repo")

import numpy as np
from contextlib import ExitStack

import concourse.bass as bass
import concourse.bacc as bacc
import concourse.tile as tile
from concourse import bass_utils
from concourse import mybir
from concourse.library_config import mlp

PADVAL = 200.0
GMAX = 8  # max 128-idx chunks per dma_gather call (HW-validated at 1024 idx)
F = 4    # dest blocks per supergroup (gather runs merge across blocks)


class Config:
    def __init__(self, n=100000, in_dim=256, hid=128, out_dim=64, ncore=8, nbank=4):
        self.N = n
        self.IN = in_dim
        self.HID = hid
        self.OUT = out_dim
        self.NCORE = ncore
        self.NPC = n // ncore
        assert self.NPC * ncore == n
        self.NB = (self.NPC + 127) // 128
        self.NPP = self.NB * 128
        self.NT = ncore * self.NPP
        self.NBANK = nbank
        assert self.NT % nbank == 0
        self.RPB = self.NT // nbank
        assert self.RPB <= 32767
        self.KT = in_dim // 128
        assert self.KT * 128 == in_dim


CFG = Config()


def prep_edges(cfg, edge_index):
    """Bucket edges by (dest core, dest block, src bank); pad each bucket to a
    multiple of 128. Returns per-core int16 gather indices (16-wrapped layout
    for dma_gather), per-core fp16 dest-row-in-block table, and the per-cell
    chunk counts (uniform across cores)."""
    c = cfg
    row = np.asarray(edge_index[0], dtype=np.int64)
    col = np.asarray(edge_index[1], dtype=np.int64)
    core = row // c.NPC
    rl = row - core * c.NPC
    blk = rl // 128
    rib = (rl % 128).astype(np.float16)
    tcol = (col // c.NPC) * c.NPP + (col % c.NPC)
    bank = tcol // c.RPB
    tloc = (tcol - bank * c.RPB).astype(np.int16)

    key = (core * c.NB + blk) * c.NBANK + bank
    order = np.argsort(key, kind="stable")
    key_s = key[order]
    tloc_s = tloc[order]
    rib_s = rib[order]
    ncell = c.NCORE * c.NB * c.NBANK
    counts = np.bincount(key_s, minlength=ncell).reshape(c.NCORE, c.NB, c.NBANK)
    starts = np.concatenate([[0], np.cumsum(counts.reshape(-1))]).reshape(-1)

    chunks = -(-counts.max(axis=0) // 128)  # (NB, NBANK) chunks per cell
    nch = int(chunks.sum())
    cw = nch * 8  # colidx columns (per chunk: 128 idx 16-wrapped -> 8 cols)

    colidx = np.zeros((c.NCORE, 128, cw), np.int16)
    rowloc = np.full((c.NCORE, 128, nch), PADVAL, np.float16)

    # chunk order: supergroup-major, bank-major within group, block within
    # bank -- so each (group, bank) run is contiguous and gather calls can
    # span block boundaries.
    qoff = np.zeros((c.NB, c.NBANK), np.int64)  # chunk offset of each cell
    q = 0
    for g0 in range(0, c.NB, F):
        for k in range(c.NBANK):
            for b in range(g0, min(g0 + F, c.NB)):
                qoff[b, k] = q
                q += chunks[b, k]

    for cc in range(c.NCORE):
        for b in range(c.NB):
            for k in range(c.NBANK):
                nk = int(chunks[b, k])
                if nk == 0:
                    continue
                ki = (cc * c.NB + b) * c.NBANK + k
                s, e = starts[ki], starts[ki + 1]
                cnt = e - s
                cap = nk * 128
                tl = np.zeros(cap, np.int16)
                tl[:cnt] = tloc_s[s:e]
                rb = np.full(cap, PADVAL, np.float16)
                rb[:cnt] = rib_s[s:e]
                qo = int(qoff[b, k])
                colidx[cc][:, qo * 8:(qo + nk) * 8] = np.tile(
                    tl.reshape(-1, 16).T, (8, 1))
                rowloc[cc][:, qo:qo + nk] = rb.reshape(nk, 128).T
    return colidx, rowloc, chunks, qoff, nch, cw


def build(cfg, chunks, qoff, nch, cw):
    c = cfg
    nc = bacc.Bacc(None, target_bir_lowering=False, debug=False,
                   num_devices=c.NCORE, name="gcn", num_swdge_queues=4)
    f16, f32, i16 = mybir.dt.float16, mybir.dt.float32, mybir.dt.int16

    xT = nc.dram_tensor("xT", (c.IN, c.NPP), f16, kind="ExternalInput")
    w1 = nc.dram_tensor("w1", (c.IN, c.HID), f16, kind="ExternalInput")
    w2p = nc.dram_tensor("w2p", (c.HID, 128), f16, kind="ExternalInput")
    iota = nc.dram_tensor("iota", (128, 128), f16, kind="ExternalInput")
    ident = nc.dram_tensor("ident", (128, 128), f16, kind="ExternalInput")
    colidx = nc.dram_tensor("colidx", (128, cw), i16, kind="ExternalInput")
    rowloc = nc.dram_tensor("rowloc", (128, nch), f16, kind="ExternalInput")
    out = nc.dram_tensor("out", (c.NPC, c.OUT), f32, kind="ExternalOutput")

    sup1 = nc.dram_tensor("sup1", (c.NPP, c.HID), f16, kind="Internal")
    tab1 = nc.dram_tensor("tab1", (c.NT, c.HID), f16, kind="Internal",
                          addr_space="Shared")
    sup2 = nc.dram_tensor("sup2", (c.NPP, 128), f16, kind="Internal")
    tab2 = nc.dram_tensor("tab2", (c.NT, 128), f16, kind="Internal",
                          addr_space="Shared")

    groups = [list(range(c.NCORE))]
    iseq = mybir.AluOpType.is_equal

    with ExitStack() as ctx:
        tc = ctx.enter_context(tile.TileContext(nc))
        nc.gpsimd.load_library(mlp)
        cpool = ctx.enter_context(tc.tile_pool(name="const", bufs=1))
        iota_sb = cpool.tile((128, 128), f16, tag="iota")
        nc.sync.dma_start(iota_sb[:], iota[:])
        ident_sb = cpool.tile((128, 128), f16, tag="ident")
        nc.sync.dma_start(ident_sb[:], ident[:])
        w2_sb = cpool.tile((c.HID, 128), f16, tag="w2")
        nc.sync.dma_start(w2_sb[:], w2p[:])
        rowloc_sb = cpool.tile((128, nch), f16, tag="rowloc")
        nc.sync.dma_start(rowloc_sb[:], rowloc[:])

        # Phase A: support1 = X @ W1, per dest block
        with tc.tile_pool(name="pa", bufs=1) as pa, \
             tc.tile_pool(name="pas", bufs=3) as pas, \
             tc.tile_pool(name="psa", bufs=2, space="PSUM") as psa:
            xk = []
            w1k = []
            for k in range(c.KT):
                t = pa.tile((128, c.NPP), f16, tag=f"x{k}", name=f"xk{k}")
                nc.sync.dma_start(t[:], xT.ap()[k * 128:(k + 1) * 128, :])
                xk.append(t)
                t = pa.tile((128, c.HID), f16, tag=f"w{k}", name=f"w1k{k}")
                nc.sync.dma_start(t[:], w1.ap()[k * 128:(k + 1) * 128, :])
                w1k.append(t)
            for b in range(c.NB):
                ps = psa.tile((128, c.HID), f32, space="PSUM")
                for k in range(c.KT):
                    nc.tensor.matmul(ps[:], xk[k][:, b * 128:(b + 1) * 128],
                                     w1k[k][:], start=(k == 0),
                                     stop=(k == c.KT - 1))
                s1 = pas.tile((128, c.HID), f16)
                nc.vector.tensor_copy(s1[:], ps[:])
                nc.sync.dma_start(sup1.ap()[b * 128:(b + 1) * 128, :], s1[:])

        nc.gpsimd.collective_compute(
            "AllGather", mybir.AluOpType.bypass, replica_groups=groups,
            ins=[sup1.ap()], outs=[tab1.ap()])

        def scatter_layer(tab, width, emit):
            """Gather+scatter all dest blocks from table `tab` (row width
            `width` fp16); call emit(b, psum_ap) per finished block.
            Processes F blocks per supergroup; gather calls merge chunks
            across blocks within each bank run."""
            gq = [0]
            with tc.tile_pool(name="poh", bufs=3) as poh, \
                 tc.tile_pool(name="pg", bufs=10) as pg, \
                 tc.tile_pool(name="pix", bufs=4) as pix, \
                 tc.tile_pool(name="pso", bufs=1, space="PSUM") as pso:
                for g0 in range(0, c.NB, F):
                    bs = list(range(g0, min(g0 + F, c.NB)))
                    wg = int(chunks[bs].sum())
                    qg0 = int(qoff[bs[0], 0])
                    # chunk index (group-relative) -> dest block
                    cblk = [b for k in range(c.NBANK) for b in bs
                            for _ in range(int(chunks[b, k]))]
                    first = {}
                    last = {}
                    for ci, b in enumerate(cblk):
                        first.setdefault(b, ci)
                        last[b] = ci
                    psb = {b: pso.tile((128, width), f32, space="PSUM",
                                       tag=f"ps{b - g0}", name=f"ps{b - g0}")
                           for b in bs}
                    for b in bs:
                        if b not in first:
                            zt = pg.tile((128, width), f16)
                            nc.vector.memset(zt[:], 0.0)
                            nc.tensor.matmul(psb[b][:], ident_sb[:], zt[:],
                                             start=True, stop=True)
                    if wg > 0:
                        oh = poh.tile((128, wg, 128), f16)
                        nc.vector.tensor_tensor(
                            out=oh[:],
                            in0=rowloc_sb[:, qg0:qg0 + wg].unsqueeze(2)
                                .to_broadcast((128, wg, 128)),
                            in1=iota_sb[:].unsqueeze(1)
                                .to_broadcast((128, wg, 128)),
                            op=iseq)
                        idx_grp = pix.tile((128, wg * 8), i16)
                        nc.sync.dma_start(
                            idx_grp[:], colidx.ap()[:, qg0 * 8:(qg0 + wg) * 8])
                        pos = 0
                        for k in range(c.NBANK):
                            run = [b for b in bs
                                   for _ in range(int(chunks[b, k]))]
                            for s0 in range(0, len(run), GMAX):
                                sn = min(GMAX, len(run) - s0)
                                off = (pos + s0) * 8
                                gt = pg.tile((128, sn, 128), f16)
                                nc.gpsimd.dma_gather(
                                    gt[:], tab.ap()[k * c.RPB:(k + 1) * c.RPB],
                                    idx_grp[:, off:off + sn * 8],
                                    sn * 128, sn * 128, 128,
                                    queue_num=gq[0] % 4)
                                gq[0] += 1
                                for j in range(sn):
                                    ci = pos + s0 + j
                                    b = cblk[ci]
                                    nc.tensor.matmul(
                                        psb[b][:], oh[:, ci, :],
                                        gt[:, j, 0:width],
                                        start=(ci == first[b]),
                                        stop=(ci == last[b]))
                            pos += len(run)
                    for b in bs:
                        emit(b, psb[b])

        # Phase C: layer-1 scatter; per block: relu, transpose, @W2 -> sup2
        with tc.tile_pool(name="ph1", bufs=3) as ph1, \
             tc.tile_pool(name="pds", bufs=3) as pds, \
             tc.tile_pool(name="pst", bufs=2, space="PSUM") as pst, \
             tc.tile_pool(name="psd", bufs=2, space="PSUM") as psd:
            def emit1(b, ps):
                h1b = ph1.tile((128, c.HID), f16, tag="h1b", name="h1b")
                nc.vector.tensor_scalar_max(h1b[:], ps[:], 0.0)
                tp = pst.tile((c.HID, 128), f16, space="PSUM")
                nc.tensor.transpose(out=tp[:], in_=h1b[:],
                                    identity=ident_sb[:])
                h1t = ph1.tile((c.HID, 128), f16, tag="h1t", name="h1t")
                nc.vector.tensor_copy(h1t[:], tp[:])
                ps2 = psd.tile((128, 128), f32, space="PSUM")
                nc.tensor.matmul(ps2[:], h1t[:], w2_sb[:],
                                 start=True, stop=True)
                s2 = pds.tile((128, 128), f16)
                nc.vector.tensor_copy(s2[:], ps2[:])
                nc.sync.dma_start(sup2.ap()[b * 128:(b + 1) * 128, :], s2[:])
            scatter_layer(tab1, c.HID, emit1)

        nc.gpsimd.collective_compute(
            "AllGather", mybir.AluOpType.bypass, replica_groups=groups,
            ins=[sup2.ap()], outs=[tab2.ap()])

        # Phase E: layer-2 scatter -> output
        with tc.tile_pool(name="po", bufs=3) as po:
            def emit2(b, ps):
                ob = po.tile((128, c.OUT), f32)
                nc.vector.tensor_copy(ob[:], ps[:])
                rows = min(128, c.NPC - b * 128)
                nc.sync.dma_start(out.ap()[b * 128:b * 128 + rows, :],
                                  ob[0:rows, :])
            scatter_layer(tab2, c.OUT, emit2)

    nc.compile()
    return nc


def make_inputs(cfg, features, edge_index, W1, W2):
    c = cfg
    colidx, rowloc, chunks, qoff, nch, cw = prep_edges(cfg, edge_index)
    iota2d = np.broadcast_to(np.arange(128, dtype=np.float16),
                             (128, 128)).copy()
    ident = np.eye(128, dtype=np.float16)
    w1 = np.ascontiguousarray(np.asarray(W1, np.float16))
    w2p = np.zeros((c.HID, 128), np.float16)
    w2p[:, :c.OUT] = np.asarray(W2, np.float16)
    in_maps = []
    for cc in range(c.NCORE):
        xc = np.asarray(features[cc * c.NPC:(cc + 1) * c.NPC], np.float32)
        xt = np.zeros((c.IN, c.NPP), np.float16)
        xt[:, :c.NPC] = xc.T.astype(np.float16)
        in_maps.append({
            "xT": np.ascontiguousarray(xt),
            "w1": w1, "w2p": w2p, "iota": iota2d, "ident": ident,
            "colidx": np.ascontiguousarray(colidx[cc]),
            "rowloc": np.ascontiguousarray(rowloc[cc]),
        })
    return in_maps, chunks, qoff, nch, cw


_LAST_NC = None


def kernel(features, edge_index, W1, W2):
    global _LAST_NC
    cfg = CFG
    in_maps, chunks, qoff, nch, cw = make_inputs(
        cfg, features, edge_index, W1, W2)
    nc = build(cfg, chunks, qoff, nch, cw)
    _LAST_NC = nc
    res = bass_utils.run_bass_kernel_spmd(
        nc, in_maps, core_ids=list(range(cfg.NCORE)))
    return np.concatenate(
        [res.results[cc]["out"] for cc in range(cfg.NCORE)], axis=0)

